# revision 20
# baseline (speedup 1.0000x reference)
"""BiGRU (B=128, T=512, D=H=512) on 8 TRN2 NeuronCores.

Strategy:
  - 8 cores, homogeneous SPMD program. Cores 0-3 run the FORWARD GRU on batch
    slices [0:32],[32:64],[64:96],[96:128]; cores 4-7 run the BACKWARD GRU on
    the same slices with x time-reversed host-side (a backward scan over x is
    a forward scan over flipped x).
  - Per core, two phases:
      Phase 1: gi = x @ w_ih.T + (b_ih + [b_hh_r, b_hh_z, 0]) for all T,
               computed as one big matmul in transposed layout
               gi.T [3H, T*B] and spilled to a DRAM scratch (f32).
      Phase 2: the sequential scan. Layout: everything transposed -- hidden
               (3H resp. H) chunks of 128 on partitions, batch on free dim.
               Per step: gh.T = w_hh.T-tiles (stationary, bf16) x h.T (moving)
               into PSUM as three groups (r, n, z), then gates on DVE/ACT.
  - Output per core: final h [128p, 4*32] (h[p, k*32+b] = h_state[b, 128k+p]),
    host reassembles [128, 1024] = concat(forward H, backward H).
"""

import os
import numpy as np
import ml_dtypes

B, T, D, H = 128, 512, 512, 512
NCORES = 8
BL = B // 4            # 32 batch rows per core
KD = D // 128          # 4 contraction tiles (input dim)
KH = H // 128          # 4 contraction tiles (hidden dim)
M3 = 3 * H // 128      # 12 output row-tiles of gi/gh
MH = H // 128          # 4 row-tiles per gate
CH = 512               # phase-1 / prefetch column chunk (one psum bank of f32)
TCH = CH // BL         # 16 timesteps per prefetch chunk

_BF16 = ml_dtypes.bfloat16

_CACHE = {}


def _build(t_steps=T):
    """Build + bacc-compile the SPMD program for one core. Cached."""
    import concourse.bass as bass  # noqa: F401
    import concourse.mybir as mybir
    import concourse.tile as tile
    from concourse import bacc
    from contextlib import ExitStack

    f32 = mybir.dt.float32
    bf16 = mybir.dt.bfloat16
    NTB = t_steps * BL
    nch = NTB // CH
    assert nch * CH == NTB

    nc = bacc.Bacc("TRN2", target_bir_lowering=False, debug=False,
                   num_devices=NCORES)

    xT = nc.dram_tensor("xT", [D, NTB], bf16, kind="ExternalInput")
    wihT = nc.dram_tensor("wihT", [D, 3 * H], bf16, kind="ExternalInput")
    whhT = nc.dram_tensor("whhT", [H, 3 * H], bf16, kind="ExternalInput")
    giB = nc.dram_tensor("giB", [128, M3], f32, kind="ExternalInput")
    bhhn = nc.dram_tensor("bhhn", [128, MH * BL], bf16, kind="ExternalInput")
    out_t = nc.dram_tensor("out", [128, MH * BL], f32, kind="ExternalOutput")
    ident_dram = nc.inline_tensor(np.eye(128, dtype=_BF16), name="ident128")

    Sig = mybir.ActivationFunctionType.Sigmoid
    Tanh = mybir.ActivationFunctionType.Tanh
    IdentF = mybir.ActivationFunctionType.Identity

    with tile.TileContext(nc) as tc, ExitStack() as ctx:
        wpool = ctx.enter_context(tc.tile_pool(name="wpool", bufs=1))
        whh_sb = wpool.tile([128, KH * 3 * H], bf16, tag="whh")
        wih_sb = wpool.tile([128, KD * 3 * H], bf16, tag="wih")
        for k in range(KH):
            nc.sync.dma_start(out=whh_sb[:, k * 3 * H:(k + 1) * 3 * H],
                              in_=whhT[k * 128:(k + 1) * 128, :])
        for k in range(KD):
            nc.sync.dma_start(out=wih_sb[:, k * 3 * H:(k + 1) * 3 * H],
                              in_=wihT[k * 128:(k + 1) * 128, :])
        giB_sb = wpool.tile([128, M3], f32, tag="giB")
        nc.sync.dma_start(out=giB_sb[:], in_=giB[:, :])
        bhhn_sb = wpool.tile([128, MH, BL], bf16, tag="bhhn")
        nc.sync.dma_start(
            out=bhhn_sb[:],
            in_=bhhn[:, :].rearrange("p (m b) -> p m b", m=MH))
        ident_sb = wpool.tile([128, 128], bf16, tag="ident")
        nc.sync.dma_start(out=ident_sb[:], in_=ident_dram[:, :])

        # gi production (the old "phase 1") is fused into the scan: the
        # N=512 x-side matmuls for chunk c+2 are drip-fed into each scan
        # step's PE stall window and evacuated PSUM -> SBUF gi tile directly.
        xq = ctx.enter_context(tc.tile_pool(name="xq", bufs=3))
        gipool = ctx.enter_context(tc.tile_pool(name="gipool", bufs=4))
        hpool = ctx.enter_context(tc.tile_pool(name="hpool", bufs=2))
        spool = ctx.enter_context(tc.tile_pool(name="spool", bufs=2))
        pxpool = ctx.enter_context(
            tc.tile_pool(name="pxpool", bufs=2, space="PSUM"))
        p2pool = ctx.enter_context(
            tc.tile_pool(name="p2pool", bufs=1, space="PSUM"))

        gic_tiles = {}

        def px_tasks(c):
            """Generator of gi-production work for chunk c."""
            xt = xq.tile([128, KD, CH], bf16, tag="xt", name=f"xt{c}")
            for k in range(KD):
                nc.sync.dma_start(
                    out=xt[:, k, :],
                    in_=xT[k * 128:(k + 1) * 128, c * CH:(c + 1) * CH])
            gt = gipool.tile([128, M3, CH], bf16, tag="gic", name=f"gic{c}")
            gic_tiles[c] = gt
            for m in range(M3):
                ps1 = pxpool.tile([128, CH], f32, tag="px", name=f"px{c}_{m}")
                for k in range(KD):
                    yield ("mm", ps1, m, k, xt)
                yield ("evac", ps1, m, gt)

        def run_px(gen, n_mm):
            """Emit up to n_mm px matmuls (evacs are free); False if done."""
            while n_mm > 0:
                ev = next(gen, None)
                if ev is None:
                    return False
                if ev[0] == "mm":
                    _, ps1, m, k, xt = ev
                    nc.tensor.matmul(
                        ps1[:],
                        lhsT=wih_sb[:, k * 3 * H + 128 * m:
                                    k * 3 * H + 128 * (m + 1)],
                        rhs=xt[:, k, :],
                        start=(k == 0), stop=(k == KD - 1))
                    n_mm -= 1
                else:
                    _, ps1, m, gt = ev
                    if m % 2 == 0:
                        nc.vector.tensor_scalar_add(
                            out=gt[:, m, :], in0=ps1[:],
                            scalar1=giB_sb[:, m:m + 1])
                    else:
                        nc.scalar.activation(
                            out=gt[:, m, :], in_=ps1[:], func=IdentF,
                            bias=giB_sb[:, m:m + 1], scale=1.0)
            return True

        for c in range(min(2, nch)):
            run_px(px_tasks(c), 10 ** 9)
        px_gen = None
        px_next = 2

        # h state kept in bf16, split in two half-tiles (k=0,1 | k=2,3) so the
        # second half's update can overlap the next step's first matmuls.
        h01 = hpool.tile([128, 2, BL], bf16, tag="h01")
        h23 = hpool.tile([128, 2, BL], bf16, tag="h23")
        nc.vector.memset(h01[:], 0.0)
        nc.vector.memset(h23[:], 0.0)

        def h_rhs(k):
            return h01[:, k, :] if k < 2 else h23[:, k - 2, :]

        for t in range(t_steps):
            if t % TCH == 0:
                if px_next < nch:
                    px_gen = px_tasks(px_next)
                    px_next += 1
                gic = gic_tiles[t // TCH]
            off = (t % TCH) * BL

            # Six PSUM banks, one per (gate, half): halves are h-rows k=0,1 vs
            # k=2,3.  Each bank is one accumulation group: the identity matmul
            # seeds the additive term (gi / b_hh_n) and starts the group; the
            # W_hh matmuls accumulate in any order; the last one stops it.
            ps = {}
            for gate, half in ((0, 0), (2, 0), (1, 0), (0, 1), (2, 1), (1, 1)):
                ps[(gate, half)] = p2pool.tile([128, 2, BL], f32,
                                               tag=f"ps{gate}{half}",
                                               name=f"ps{gate}{half}")

            def ident_mm(gate, half):
                if gate == 2:
                    rhs = bhhn_sb[:, 2 * half:2 * half + 2, :]
                else:
                    m0 = gate * MH + 2 * half
                    rhs = gic[:, m0:m0 + 2, off:off + BL]
                nc.tensor.matmul(ps[(gate, half)][:], lhsT=ident_sb[:],
                                 rhs=rhs, start=True, stop=False)

            def wmm(gate, half, ks, stop=False):
                for i in range(2):
                    mg = gate * MH + 2 * half + i
                    for j, k in enumerate(ks):
                        nc.tensor.matmul(
                            ps[(gate, half)][:, i, :],
                            lhsT=whh_sb[:, k * 3 * H + 128 * mg:
                                        k * 3 * H + 128 * (mg + 1)],
                            rhs=h_rhs(k),
                            start=False,
                            stop=(stop and i == 1 and j == len(ks) - 1))

            # PE stream: half-0 banks finish early (their k23 right after the
            # k01 phase), half-1 k01 keeps the PE busy while half-0's gate
            # chain runs; next step's k01 only needs h01.
            ident_mm(0, 0); wmm(0, 0, (0, 1))
            ident_mm(2, 0); wmm(2, 0, (0, 1))
            ident_mm(1, 0); wmm(1, 0, (0, 1))
            wmm(0, 0, (2, 3), stop=True)
            wmm(2, 0, (2, 3), stop=True)
            wmm(1, 0, (2, 3), stop=True)
            ident_mm(0, 1); wmm(0, 1, (0, 1))
            ident_mm(2, 1); wmm(2, 1, (0, 1))
            ident_mm(1, 1); wmm(1, 1, (0, 1))
            wmm(0, 1, (2, 3), stop=True)
            wmm(2, 1, (2, 3), stop=True)
            wmm(1, 1, (2, 3), stop=True)

            # Per-half gate chain: r = sig(ps_r); n = tanh(gi_n + r*ps_n);
            # h' = z*h + (1-z)*n with z = sig(ps_z), 1-z = sig(-ps_z).
            # a = z*h runs off the critical chain; after tanh only two DVE
            # ops (b = zb*n; h' = a + b) remain.
            h01_new = hpool.tile([128, 2, BL], bf16, tag="h01")
            h23_new = hpool.tile([128, 2, BL], bf16, tag="h23")
            for half, (h_old, h_new) in enumerate(((h01, h01_new),
                                                   (h23, h23_new))):
                m0 = 2 * MH + 2 * half
                r = spool.tile([128, 2, BL], f32, tag=f"r{half}")
                nc.scalar.activation(out=r[:], in_=ps[(0, half)][:], func=Sig)
                z = spool.tile([128, 2, BL], f32, tag=f"z{half}")
                nc.scalar.activation(out=z[:], in_=ps[(1, half)][:], func=Sig)
                zb = spool.tile([128, 2, BL], f32, tag=f"zb{half}")
                nc.scalar.activation(out=zb[:], in_=ps[(1, half)][:],
                                     func=Sig, scale=-1.0)
                a = spool.tile([128, 2, BL], f32, tag=f"a{half}")
                nc.vector.tensor_mul(out=a[:], in0=z[:], in1=h_old[:])
                nt = spool.tile([128, 2, BL], f32, tag=f"nt{half}")
                nc.vector.tensor_mul(out=nt[:], in0=ps[(2, half)][:], in1=r[:])
                nc.vector.tensor_add(out=nt[:], in0=nt[:],
                                     in1=gic[:, m0:m0 + 2, off:off + BL])
                n_ = spool.tile([128, 2, BL], f32, tag=f"n{half}")
                nc.scalar.activation(out=n_[:], in_=nt[:], func=Tanh)
                b = spool.tile([128, 2, BL], f32, tag=f"b{half}")
                nc.vector.tensor_mul(out=b[:], in0=zb[:], in1=n_[:])
                nc.vector.tensor_add(out=h_new[:], in0=a[:], in1=b[:])
            h01, h23 = h01_new, h23_new

            if px_gen is not None and not run_px(px_gen, 3):
                px_gen = None

        hf = spool.tile([128, KH, BL], f32, tag="hf")
        nc.scalar.copy(out=hf[:, 0:2, :], in_=h01[:])
        nc.scalar.copy(out=hf[:, 2:4, :], in_=h23[:])
        nc.sync.dma_start(
            out=out_t[:, :].rearrange("p (k b) -> p k b", k=KH), in_=hf[:])

    nc.compile()
    return nc


def _host_inputs(x, w_ih_f, w_hh_f, b_ih_f, b_hh_f,
                 w_ih_b, w_hh_b, b_ih_b, b_hh_b, t_steps=T):
    """Per-core input dicts."""
    x = np.asarray(x, np.float32)

    def direction(w_ih, w_hh, b_ih, b_hh):
        wihT = np.ascontiguousarray(np.asarray(w_ih, np.float32).T).astype(_BF16)
        whhT = np.ascontiguousarray(np.asarray(w_hh, np.float32).T).astype(_BF16)
        b_ih = np.asarray(b_ih, np.float32)
        b_hh = np.asarray(b_hh, np.float32)
        gib = b_ih.copy()
        gib[:2 * H] += b_hh[:2 * H]
        giB = np.ascontiguousarray(gib.reshape(M3, 128).T)            # [128,12]
        bn = b_hh[2 * H:].reshape(KH, 128).T                           # [128,4]
        bhhn = np.ascontiguousarray(
            np.broadcast_to(bn[:, :, None], (128, KH, BL))
        ).reshape(128, KH * BL).astype(_BF16)
        return wihT, whhT, giB, bhhn

    fwd = direction(w_ih_f, w_hh_f, b_ih_f, b_hh_f)
    bwd = direction(w_ih_b, w_hh_b, b_ih_b, b_hh_b)

    in_maps = []
    for core in range(NCORES):
        is_fwd = core < 4
        bs = (core % 4) * BL
        xs = x[bs:bs + BL, :t_steps, :]
        if not is_fwd:
            xs = xs[:, ::-1, :]
        xTn = np.ascontiguousarray(xs.transpose(2, 1, 0)).reshape(
            D, t_steps * BL).astype(_BF16)
        wihT, whhT, giB, bhhn = fwd if is_fwd else bwd
        in_maps.append({"xT": xTn, "wihT": wihT, "whhT": whhT,
                        "giB": giB, "bhhn": bhhn})
    return in_maps


def _assemble(results):
    out = np.empty((B, 2 * H), np.float32)
    for core in range(NCORES):
        o = np.asarray(results[core]["out"], np.float32)
        hmat = o.reshape(128, KH, BL).transpose(2, 1, 0).reshape(BL, H)
        bs = (core % 4) * BL
        if core < 4:
            out[bs:bs + BL, :H] = hmat
        else:
            out[bs:bs + BL, H:] = hmat
    return out


LAST_RESULTS = None


def kernel(x, w_ih_f, w_hh_f, b_ih_f, b_hh_f,
           w_ih_b, w_hh_b, b_ih_b, b_hh_b):
    global LAST_RESULTS
    from concourse.bass_utils import run_bass_kernel_spmd

    if "nc" not in _CACHE:
        _CACHE["nc"] = _build(T)
    nc = _CACHE["nc"]

    in_maps = _host_inputs(x, w_ih_f, w_hh_f, b_ih_f, b_hh_f,
                           w_ih_b, w_hh_b, b_ih_b, b_hh_b)
    trace = os.environ.get("KERNEL_TRACE", "0") == "1"
    kwargs = {}
    if trace:
        kwargs = dict(trace=True, tmpdir=os.environ.get("KERNEL_TRACE_DIR"))
    res = run_bass_kernel_spmd(nc, in_maps, list(range(NCORES)), **kwargs)
    LAST_RESULTS = res
    return _assemble(res.results)
